# revision 2
# baseline (speedup 1.0000x reference)
"""DYSPN attention-conv kernel v2 for Trainium2 (8 NeuronCores, batch-parallel).

Math (unfold/fold pair collapses algebraically; see derivation):
  per image, tap k=(i,j) != center, ring r = INDEX[i,j], dy = 3-i, dx = 3-j:
    z_k[y,x] = att_r[y,x] * aff_k[y,x]
    U[y,x]   = sum_k z_k[y,x]
    D[y,x]   = sum_k (|z_k| - z_k)[y,x] = sum_k 2*relu(-z_k)   (att >= 0)
    T[y,x]   = sum_k z_k[y+dy, x+dx]  (zero outside image)
  out = ((T + att3)*cs + D*co) / (U + D + att3 + eps)
  (identical to the reference: A = U + D, denom = A + att3 + eps,
   out = (T+att3)*cs/denom + ((denom - U - att3)/denom)*co, and
   denom - U - att3 = D + eps)

Implementation strategy (per core: 2 images, 4 blocks of 128 rows):
  - Host: cast aff/att/cs to bf16; pack aff as guard-padded planes
    [img, blk, part(row), tap(ring-sorted), 264] so DMA lands compute-ready
    (no transposes, no guard memsets).  HBM traffic halves vs fp32.
  - DVE: z = att*aff (bf16 TT 2x, 3 ring-broadcast instrs/block), pre-fold
    of ring0 halves for the U and D reductions, fp32 epilogue.
  - ACT/POOL/DVE split: d = relu(-z) planes (tensor_scalar(min 0, mult -1) /
    activation(Relu, scale=-1)).
  - PE: U/D/T reductions as bf16 banded-identity matmuls, 2 taps per matmul
    into 2-wide PSUM accumulators [128,2,256] (one bank), folded in the
    epilogue.  T row-shifts via band-diagonal offsets, column shifts baked
    into custom moving APs (stride 264 + dx2-dx1); cross-block halo rows via
    off-diagonal band windows.
"""
import sys

sys.path.insert(0, "/opt/trn_rl_repo")

import numpy as np
import ml_dtypes

import concourse.bass as bass  # noqa: F401
import concourse.tile as tile
from concourse import bacc, mybir
from concourse.ap import AP
from concourse.bass_utils import run_bass_kernel_spmd

BF16 = mybir.dt.bfloat16
FP32 = mybir.dt.float32

N_CORES = 8
B_FULL = 16
B_CORE = B_FULL // N_CORES
H = W = 256
K = 7
GW = 4                    # zero guard cols each side (host-packed)
PW = W + 2 * GW           # 264: plane pitch in the z tile
NTAP = 48
BANDW = 390
C0 = 131                  # identity diagonal column offset in band1
ID2 = BANDW               # ident2 (2.0 diagonal) starts at this column
EPS = 1e-6

_INDEX = np.array([0, 0, 0, 0, 0, 0, 0,
                   0, 1, 1, 1, 1, 1, 0,
                   0, 1, 2, 2, 2, 1, 0,
                   0, 1, 2, 3, 2, 1, 0,
                   0, 1, 2, 2, 2, 1, 0,
                   0, 1, 1, 1, 1, 1, 0,
                   0, 0, 0, 0, 0, 0, 0], dtype=np.int64).reshape(K, K)

# ring-major, row-minor, col-minor tap order; t = SBUF/DRAM plane index
TAPORD = [(i, j) for r in (0, 1, 2) for i in range(K) for j in range(K)
          if (i, j) != (3, 3) and _INDEX[i, j] == r]
RING_SEG = [(0, 24, 0), (24, 40, 1), (40, 48, 2)]   # t-ranges per ring
# mult instr ranges (ring0 split at row boundary t=11 for DMA overlap)
MULT_RUNS = [(0, 11, 0), (11, 24, 0), (24, 40, 1), (40, 48, 2)]
DMA_CHUNKS = [(0, 11), (11, 24), (24, 40), (40, 48)]

# T groups: maximal t-contiguous runs sharing row i (same dy)
TGROUPS = []  # (t_lo, t_hi, i)
for t, (i, j) in enumerate(TAPORD):
    if TGROUPS and TGROUPS[-1][2] == i and TGROUPS[-1][1] == t:
        TGROUPS[-1][1] = t + 1
    else:
        TGROUPS.append([t, t + 1, i])
TGROUPS = [tuple(g) for g in TGROUPS]


def dxof(t):
    return 3 - TAPORD[t][1]


def dyof(t):
    return 3 - TAPORD[t][0]


def _chunk_of(t):
    for ci, (lo, hi) in enumerate(DMA_CHUNKS):
        if lo <= t < hi:
            return ci
    raise AssertionError


def band_np() -> np.ndarray:
    b = np.zeros((128, BANDW + 128), dtype=np.float32)
    for p in range(128):
        b[p, p + C0] = 1.0
        b[p, ID2 + p] = 2.0
    return b.astype(ml_dtypes.bfloat16)


def _to_bf16_round(x: np.ndarray) -> np.ndarray:
    """fp32 -> bf16 with round-to-nearest-even, fast numpy path."""
    u = x.view(np.uint32)
    r = ((u >> 16) & 1).astype(np.uint32)
    out = ((u + 0x7FFF + r) >> 16).astype(np.uint16)
    return out.view(ml_dtypes.bfloat16)


def pack_inputs(aff, att, cs, co):
    """Full fp32 inputs -> per-core input maps (host-side layout/cast only)."""
    B = B_FULL
    # affinity: [B,49,H,W] -> guarded bf16 planes [B,2,128,48,264] in TAPORD
    kidx = np.array([i * K + j for (i, j) in TAPORD])
    aff_sel = aff[:, kidx]                             # [B,48,H,W]
    aff_bf = _to_bf16_round(np.ascontiguousarray(aff_sel))
    packed = np.zeros((B, 2, 128, NTAP, PW), dtype=ml_dtypes.bfloat16)
    a = aff_bf.reshape(B, NTAP, 2, 128, W).transpose(0, 2, 3, 1, 4)
    packed[:, :, :, :, GW:GW + W] = a
    att_bf = _to_bf16_round(np.ascontiguousarray(att))  # [B,4,H,W]
    att_p = att_bf.reshape(B, 4, 2, 128, W).transpose(0, 2, 3, 1, 4)
    att_p = np.ascontiguousarray(att_p)                 # [B,2,128,4,W]
    cs32 = np.ascontiguousarray(cs, dtype=np.float32).reshape(B, 2, 128, W)
    co32 = np.ascontiguousarray(co, dtype=np.float32).reshape(B, 2, 128, W)
    band = band_np()

    in_maps = []
    for c in range(N_CORES):
        s = slice(c * B_CORE, (c + 1) * B_CORE)
        in_maps.append({
            "aff": np.ascontiguousarray(packed[s]),
            "att": np.ascontiguousarray(att_p[s]),
            "cs": np.ascontiguousarray(cs32[s]),
            "co": np.ascontiguousarray(co32[s]),
            "band": band,
        })
    return in_maps


def _pair_ap(zt, t1, t2, w1, w2):
    """Custom moving AP over taps {t1,t2} with per-tap column windows."""
    base = zt[:]
    pstride = base.ap[0][0]
    off = base.offset + t1 * PW + w1
    stride = (t2 - t1) * PW + (w2 - w1)
    return AP(base.tensor, off, [[pstride, 128], [stride, 2], [1, W]])


def _single_ap(zt, t, w):
    base = zt[:]
    pstride = base.ap[0][0]
    return AP(base.tensor, base.offset + t * PW + w, [[pstride, 128], [1, W]])


def _build():
    nc = bacc.Bacc("TRN2", target_bir_lowering=False, debug=False,
                   num_devices=N_CORES)
    aff = nc.dram_tensor("aff", [B_CORE, 2, 128, NTAP, PW], BF16,
                         kind="ExternalInput").ap()
    att = nc.dram_tensor("att", [B_CORE, 2, 128, 4, W], BF16,
                         kind="ExternalInput").ap()
    cs = nc.dram_tensor("cs", [B_CORE, 2, 128, W], FP32,
                        kind="ExternalInput").ap()
    co = nc.dram_tensor("co", [B_CORE, 2, 128, W], FP32,
                        kind="ExternalInput").ap()
    band = nc.dram_tensor("band", [128, BANDW + 128], BF16,
                          kind="ExternalInput").ap()
    out = nc.dram_tensor("out", [B_CORE, 1, H, W], FP32,
                         kind="ExternalOutput").ap()

    with tile.TileContext(nc) as tc:
        with tc.tile_pool(name="const", bufs=1) as cpool, \
             tc.tile_pool(name="inp", bufs=4) as ipool, \
             tc.tile_pool(name="zp", bufs=4) as zpool, \
             tc.tile_pool(name="dp", bufs=2) as dpool, \
             tc.tile_pool(name="fp", bufs=2) as fpool, \
             tc.tile_pool(name="ep", bufs=2) as epool, \
             tc.tile_pool(name="ps", bufs=2, space="PSUM") as pspool:

            bandt = cpool.tile([128, BANDW + 128], BF16)
            nc.sync.dma_start(out=bandt[:], in_=band[:, :])
            ident = bandt[:, C0:C0 + 128]
            ident2 = bandt[:, ID2:ID2 + 128]

            for img in range(B_CORE):
                # ---- input DMAs ----
                zts, attfs, csts, cots = [], [], [], []
                for b in range(2):
                    attf = ipool.tile([128, 4, W], BF16, tag="attf")
                    nc.sync.dma_start(out=attf[:], in_=att[img, b])
                    cst = ipool.tile([128, W], FP32, tag="cst")
                    nc.sync.dma_start(out=cst[:], in_=cs[img, b])
                    cot = ipool.tile([128, W], FP32, tag="cot")
                    nc.sync.dma_start(out=cot[:], in_=co[img, b])
                    attfs.append(attf)
                    csts.append(cst)
                    cots.append(cot)
                    zt = zpool.tile([128, NTAP, PW], BF16, tag="zt")
                    zts.append(zt)
                for ci, (lo, hi) in enumerate(DMA_CHUNKS):
                    for b in range(2):
                        nc.sync.dma_start(out=zts[b][:, lo:hi, :],
                                          in_=aff[img, b, :, lo:hi, :])

                # ---- PSUM accumulators (one bank each) ----
                psU = [pspool.tile([128, 2, W], FP32, tag="U", name=f"psU{_b}")
                       for _b in range(2)]
                psD = [pspool.tile([128, 2, W], FP32, tag="D", name=f"psD{_b}")
                       for _b in range(2)]
                psT = [pspool.tile([128, 2, W], FP32, tag="T", name=f"psT{_b}")
                       for _b in range(2)]
                started = set()

                def mm(acc, b, stop=False, **kw):
                    key = (acc, b)
                    nc.tensor.matmul(start=(key not in started), stop=stop,
                                     **kw)
                    started.add(key)

                dts = [dpool.tile([128, NTAP, W], BF16, tag="dt",
                                   name=f"dt{_b}") for _b in range(2)]
                # ring0 pre-folds for U (z) and D (d): 24 -> 12 planes
                zus = [fpool.tile([128, 12, W], BF16, tag="zu",
                                   name=f"zu{_b}") for _b in range(2)]
                dus = [fpool.tile([128, 12, W], BF16, tag="du",
                                   name=f"du{_b}") for _b in range(2)]

                # ---- per-chunk compute ----
                for ci, (lo, hi) in enumerate(DMA_CHUNKS):
                    ring = 0 if hi <= 24 else (1 if hi <= 40 else 2)
                    for b in range(2):
                        zt = zts[b]
                        zwin = zt[:, lo:hi, GW:GW + W]
                        # z = att_r * aff  (DVE, in place, bf16 2x)
                        nc.vector.tensor_tensor(
                            out=zwin, in0=zwin,
                            in1=attfs[b][:, ring:ring + 1, :].broadcast_to(
                                [128, hi - lo, W]),
                            op=mybir.AluOpType.mult)
                        # d = relu(-z): split DVE / POOL / ACT
                        dsl = dts[b][:, lo:hi, :]
                        if ci == 0:
                            nc.vector.tensor_scalar(
                                out=dsl, in0=zwin, scalar1=0.0, scalar2=-1.0,
                                op0=mybir.AluOpType.min,
                                op1=mybir.AluOpType.mult)
                        else:
                            nc.scalar.activation(
                                dsl, zwin, mybir.ActivationFunctionType.Relu,
                                scale=-1.0)

                    for b in range(2):
                        zt = zts[b]
                        if ci == 1:
                            # ring0 folds (need chunks 0+1): U and D inputs
                            nc.vector.tensor_tensor(
                                out=zus[b][:], in0=zt[:, 0:12, GW:GW + W],
                                in1=zt[:, 12:24, GW:GW + W],
                                op=mybir.AluOpType.add)
                            nc.vector.tensor_tensor(
                                out=dus[b][:], in0=dts[b][:, 0:12, :],
                                in1=dts[b][:, 12:24, :],
                                op=mybir.AluOpType.add)
                            for h in range(6):
                                mm("U", b, out=psU[b][:], lhsT=ident,
                                   rhs=zus[b][:, 2 * h:2 * h + 2, :])
                                mm("D", b, out=psD[b][:], lhsT=ident2,
                                   rhs=dus[b][:, 2 * h:2 * h + 2, :])
                        elif ci >= 2:
                            for t in range(lo, hi, 2):
                                mm("U", b, out=psU[b][:], lhsT=ident,
                                   rhs=zt[:, t:t + 2, GW:GW + W])
                                mm("D", b, stop=(t == 46),
                                   out=psD[b][:], lhsT=ident2,
                                   rhs=dts[b][:, t:t + 2, :])
                        # T matmuls for groups inside this chunk
                        for (g_lo, g_hi, i) in TGROUPS:
                            if g_lo < lo or g_lo >= hi:
                                continue
                            dy = 3 - i
                            lw = bandt[:, C0 + dy:C0 + dy + 128]
                            t = g_lo
                            while t + 1 < g_hi:
                                mm("T", b, out=psT[b][:], lhsT=lw,
                                   rhs=_pair_ap(zt, t, t + 1,
                                                GW + dxof(t), GW + dxof(t + 1)))
                                t += 2
                            if t < g_hi:
                                mm("T", b, out=psT[b][:, 0, :], lhsT=lw,
                                   rhs=_single_ap(zt, t, GW + dxof(t)))
                            # halo: b0 rows need dy>0 from b1; b1 need dy<0;
                            # emitted at b==1 so both blocks' z is ready
                            if b == 1 and dy != 0:
                                if dy > 0:
                                    hw = bandt[:, 3 + dy:3 + dy + 128]
                                    dst, other = 0, zts[1]
                                else:
                                    hw = bandt[:, 259 + dy:259 + dy + 128]
                                    dst, other = 1, zts[0]
                                t = g_lo
                                while t + 1 < g_hi:
                                    mm("T", dst, out=psT[dst][:], lhsT=hw,
                                       rhs=_pair_ap(other, t, t + 1,
                                                    GW + dxof(t),
                                                    GW + dxof(t + 1)))
                                    t += 2
                                if t < g_hi:
                                    mm("T", dst, out=psT[dst][:, 0, :],
                                       lhsT=hw,
                                       rhs=_single_ap(other, t, GW + dxof(t)))

                # closers: psU += att3, psT += att3 (stop their groups)
                for b in range(2):
                    mm("U", b, stop=True, out=psU[b][:, 0, :], lhsT=ident,
                       rhs=attfs[b][:, 3, :])
                    mm("T", b, stop=True, out=psT[b][:, 0, :], lhsT=ident,
                       rhs=attfs[b][:, 3, :])

                # ---- epilogue ----
                for b in range(2):
                    # DVE can read only one PSUM operand per op: stage the
                    # second accumulator halves through SBUF on ACT
                    u1 = epool.tile([128, W], FP32, tag="u1")
                    nc.scalar.copy(u1[:], psU[b][:, 1, :])
                    d1 = epool.tile([128, W], FP32, tag="d1")
                    nc.scalar.copy(d1[:], psD[b][:, 1, :])
                    t1 = epool.tile([128, W], FP32, tag="t1")
                    nc.scalar.copy(t1[:], psT[b][:, 1, :])
                    # in-place accumulations to keep the tile count low
                    nc.vector.tensor_tensor(out=u1[:], in0=psU[b][:, 0, :],
                                            in1=u1[:],
                                            op=mybir.AluOpType.add)
                    nc.vector.tensor_tensor(out=d1[:], in0=psD[b][:, 0, :],
                                            in1=d1[:],
                                            op=mybir.AluOpType.add)
                    e = epool.tile([128, W], FP32, tag="e")
                    nc.vector.scalar_tensor_tensor(
                        out=e[:], in0=u1[:], scalar=EPS, in1=d1[:],
                        op0=mybir.AluOpType.add, op1=mybir.AluOpType.add)
                    rcp = epool.tile([128, W], FP32, tag="rcp")
                    nc.vector.reciprocal_approx_fast(out=rcp[:], in_=e[:])
                    nc.vector.tensor_tensor(out=d1[:], in0=d1[:],
                                            in1=cots[b][:],
                                            op=mybir.AluOpType.mult)
                    nc.vector.tensor_tensor(out=t1[:], in0=psT[b][:, 0, :],
                                            in1=t1[:],
                                            op=mybir.AluOpType.add)
                    nc.vector.tensor_tensor(out=t1[:], in0=t1[:],
                                            in1=csts[b][:],
                                            op=mybir.AluOpType.mult)
                    nc.vector.tensor_tensor(out=t1[:], in0=t1[:],
                                            in1=d1[:],
                                            op=mybir.AluOpType.add)
                    outt = epool.tile([128, W], FP32, tag="outt")
                    nc.vector.tensor_tensor(out=outt[:], in0=t1[:],
                                            in1=rcp[:],
                                            op=mybir.AluOpType.mult)
                    nc.sync.dma_start(
                        out=out[img, 0, b * 128:b * 128 + 128, :],
                        in_=outt[:])

    nc.compile()
    return nc


_NC_CACHE = None


def _get_nc():
    global _NC_CACHE
    if _NC_CACHE is None:
        _NC_CACHE = _build()
    return _NC_CACHE


def run(inputs: dict, trace: bool = False):
    aff = np.ascontiguousarray(np.asarray(inputs["affinity"], dtype=np.float32))
    att = np.ascontiguousarray(np.asarray(inputs["attention"], dtype=np.float32))
    cs = np.ascontiguousarray(
        np.asarray(inputs["current_segmentation"], dtype=np.float32))
    co = np.ascontiguousarray(
        np.asarray(inputs["coarse_segmentation"], dtype=np.float32))
    in_maps = pack_inputs(aff, att, cs, co)

    nc = _get_nc()
    last_err = None
    for attempt in range(3):
        try:
            res = run_bass_kernel_spmd(nc, in_maps, list(range(N_CORES)),
                                       trace=trace)
            break
        except Exception as e:
            last_err = e
            import time
            time.sleep(10)
    else:
        raise last_err
    full = np.concatenate([res.results[c]["out"] for c in range(N_CORES)],
                          axis=0)
    return full, res


def kernel(**inputs) -> np.ndarray:
    out, _ = run(inputs, trace=False)
    return out


# revision 3
# speedup vs baseline: 1.0633x; 1.0633x over previous
"""DYSPN attention-conv kernel v2 for Trainium2 (8 NeuronCores, batch-parallel).

Math (unfold/fold pair collapses algebraically; see derivation):
  per image, tap k=(i,j) != center, ring r = INDEX[i,j], dy = 3-i, dx = 3-j:
    z_k[y,x] = att_r[y,x] * aff_k[y,x]
    U[y,x]   = sum_k z_k[y,x]
    D[y,x]   = sum_k (|z_k| - z_k)[y,x] = sum_k 2*relu(-z_k)   (att >= 0)
    T[y,x]   = sum_k z_k[y+dy, x+dx]  (zero outside image)
  out = ((T + att3)*cs + D*co) / (U + D + att3 + eps)
  (identical to the reference: A = U + D, denom = A + att3 + eps,
   out = (T+att3)*cs/denom + ((denom - U - att3)/denom)*co, and
   denom - U - att3 = D + eps)

Implementation strategy (per core: 2 images, 4 blocks of 128 rows):
  - Host: cast aff/att/cs to bf16; pack aff as guard-padded planes
    [img, blk, part(row), tap(ring-sorted), 264] so DMA lands compute-ready
    (no transposes, no guard memsets).  HBM traffic halves vs fp32.
  - DVE: z = att*aff (bf16 TT 2x, 3 ring-broadcast instrs/block), pre-fold
    of ring0 halves for the U and D reductions, fp32 epilogue.
  - ACT/POOL/DVE split: d = relu(-z) planes (tensor_scalar(min 0, mult -1) /
    activation(Relu, scale=-1)).
  - PE: U/D/T reductions as bf16 banded-identity matmuls, 2 taps per matmul
    into 2-wide PSUM accumulators [128,2,256] (one bank), folded in the
    epilogue.  T row-shifts via band-diagonal offsets, column shifts baked
    into custom moving APs (stride 264 + dx2-dx1); cross-block halo rows via
    off-diagonal band windows.
"""
import sys

sys.path.insert(0, "/opt/trn_rl_repo")

import numpy as np
import ml_dtypes

import concourse.bass as bass  # noqa: F401
import concourse.tile as tile
from concourse import bacc, mybir
from concourse.ap import AP
from concourse.bass_utils import run_bass_kernel_spmd

BF16 = mybir.dt.bfloat16
FP32 = mybir.dt.float32

N_CORES = 8
B_FULL = 16
B_CORE = B_FULL // N_CORES
H = W = 256
K = 7
GW = 4                    # zero guard cols each side (host-packed)
PW = W + 2 * GW           # 264: plane pitch in the z tile
NTAP = 48
BANDW = 390
C0 = 131                  # identity diagonal column offset in band1
ID2 = BANDW               # ident2 (2.0 diagonal) starts at this column
EPS = 1e-6

_INDEX = np.array([0, 0, 0, 0, 0, 0, 0,
                   0, 1, 1, 1, 1, 1, 0,
                   0, 1, 2, 2, 2, 1, 0,
                   0, 1, 2, 3, 2, 1, 0,
                   0, 1, 2, 2, 2, 1, 0,
                   0, 1, 1, 1, 1, 1, 0,
                   0, 0, 0, 0, 0, 0, 0], dtype=np.int64).reshape(K, K)

# ring-major, row-minor, col-minor tap order; t = SBUF/DRAM plane index
TAPORD = [(i, j) for r in (0, 1, 2) for i in range(K) for j in range(K)
          if (i, j) != (3, 3) and _INDEX[i, j] == r]
RING_SEG = [(0, 24, 0), (24, 40, 1), (40, 48, 2)]   # t-ranges per ring
# mult instr ranges (ring0 split at row boundary t=11 for DMA overlap)
MULT_RUNS = [(0, 11, 0), (11, 24, 0), (24, 40, 1), (40, 48, 2)]
DMA_CHUNKS = [(0, 11), (11, 24), (24, 40), (40, 48)]

# T groups: maximal t-contiguous runs sharing row i (same dy)
TGROUPS = []  # (t_lo, t_hi, i)
for t, (i, j) in enumerate(TAPORD):
    if TGROUPS and TGROUPS[-1][2] == i and TGROUPS[-1][1] == t:
        TGROUPS[-1][1] = t + 1
    else:
        TGROUPS.append([t, t + 1, i])
TGROUPS = [tuple(g) for g in TGROUPS]


def dxof(t):
    return 3 - TAPORD[t][1]


def dyof(t):
    return 3 - TAPORD[t][0]


def _chunk_of(t):
    for ci, (lo, hi) in enumerate(DMA_CHUNKS):
        if lo <= t < hi:
            return ci
    raise AssertionError


def band_np() -> np.ndarray:
    b = np.zeros((128, BANDW + 128), dtype=np.float32)
    for p in range(128):
        b[p, p + C0] = 1.0
        b[p, ID2 + p] = 2.0
    return b.astype(ml_dtypes.bfloat16)


def _to_bf16_round(x: np.ndarray) -> np.ndarray:
    """fp32 -> bf16 with round-to-nearest-even, fast numpy path."""
    u = x.view(np.uint32)
    r = ((u >> 16) & 1).astype(np.uint32)
    out = ((u + 0x7FFF + r) >> 16).astype(np.uint16)
    return out.view(ml_dtypes.bfloat16)


def pack_inputs(aff, att, cs, co):
    """Full fp32 inputs -> per-core input maps (host-side layout/cast only)."""
    B = B_FULL
    # affinity: [B,49,H,W] -> guarded bf16 planes [B,2,128,48,264] in TAPORD
    kidx = np.array([i * K + j for (i, j) in TAPORD])
    aff_sel = aff[:, kidx]                             # [B,48,H,W]
    aff_bf = _to_bf16_round(np.ascontiguousarray(aff_sel))
    packed = np.zeros((B, 2, 128, NTAP, PW), dtype=ml_dtypes.bfloat16)
    a = aff_bf.reshape(B, NTAP, 2, 128, W).transpose(0, 2, 3, 1, 4)
    packed[:, :, :, :, GW:GW + W] = a
    att_bf = _to_bf16_round(np.ascontiguousarray(att))  # [B,4,H,W]
    att_p = att_bf.reshape(B, 4, 2, 128, W).transpose(0, 2, 3, 1, 4)
    att_p = np.ascontiguousarray(att_p)                 # [B,2,128,4,W]
    cs32 = np.ascontiguousarray(cs, dtype=np.float32).reshape(B, 2, 128, W)
    co32 = np.ascontiguousarray(co, dtype=np.float32).reshape(B, 2, 128, W)
    band = band_np()

    in_maps = []
    for c in range(N_CORES):
        s = slice(c * B_CORE, (c + 1) * B_CORE)
        in_maps.append({
            "aff": np.ascontiguousarray(packed[s]),
            "att": np.ascontiguousarray(att_p[s]),
            "cs": np.ascontiguousarray(cs32[s]),
            "co": np.ascontiguousarray(co32[s]),
            "band": band,
        })
    return in_maps


def _pair_ap(zt, t1, t2, w1, w2):
    """Custom moving AP over taps {t1,t2} with per-tap column windows."""
    base = zt[:]
    pstride = base.ap[0][0]
    off = base.offset + t1 * PW + w1
    stride = (t2 - t1) * PW + (w2 - w1)
    return AP(base.tensor, off, [[pstride, 128], [stride, 2], [1, W]])


def _single_ap(zt, t, w):
    base = zt[:]
    pstride = base.ap[0][0]
    return AP(base.tensor, base.offset + t * PW + w, [[pstride, 128], [1, W]])


def _build():
    nc = bacc.Bacc("TRN2", target_bir_lowering=False, debug=False,
                   num_devices=N_CORES)
    aff = nc.dram_tensor("aff", [B_CORE, 2, 128, NTAP, PW], BF16,
                         kind="ExternalInput").ap()
    att = nc.dram_tensor("att", [B_CORE, 2, 128, 4, W], BF16,
                         kind="ExternalInput").ap()
    cs = nc.dram_tensor("cs", [B_CORE, 2, 128, W], FP32,
                        kind="ExternalInput").ap()
    co = nc.dram_tensor("co", [B_CORE, 2, 128, W], FP32,
                        kind="ExternalInput").ap()
    band = nc.dram_tensor("band", [128, BANDW + 128], BF16,
                          kind="ExternalInput").ap()
    out = nc.dram_tensor("out", [B_CORE, 1, H, W], FP32,
                         kind="ExternalOutput").ap()

    with tile.TileContext(nc) as tc:
        with tc.tile_pool(name="const", bufs=1) as cpool, \
             tc.tile_pool(name="inp", bufs=4) as ipool, \
             tc.tile_pool(name="zp", bufs=4) as zpool, \
             tc.tile_pool(name="dp", bufs=2) as dpool, \
             tc.tile_pool(name="fp", bufs=2) as fpool, \
             tc.tile_pool(name="ep", bufs=2) as epool, \
             tc.tile_pool(name="ps", bufs=2, space="PSUM") as pspool:

            bandt = cpool.tile([128, BANDW + 128], BF16)
            nc.scalar.dma_start(out=bandt[:], in_=band[:, :])
            ident = bandt[:, C0:C0 + 128]
            ident2 = bandt[:, ID2:ID2 + 128]

            for img in range(B_CORE):
                # ---- input DMAs ----
                zts, attfs, csts, cots = [], [], [], []
                for b in range(2):
                    attf = ipool.tile([128, 4, W], BF16, tag="attf")
                    nc.scalar.dma_start(out=attf[:], in_=att[img, b])
                    cst = ipool.tile([128, W], FP32, tag="cst")
                    nc.scalar.dma_start(out=cst[:], in_=cs[img, b])
                    cot = ipool.tile([128, W], FP32, tag="cot")
                    nc.scalar.dma_start(out=cot[:], in_=co[img, b])
                    attfs.append(attf)
                    csts.append(cst)
                    cots.append(cot)
                    zt = zpool.tile([128, NTAP, PW], BF16, tag="zt")
                    zts.append(zt)
                for ci, (lo, hi) in enumerate(DMA_CHUNKS):
                    for b in range(2):
                        nc.sync.dma_start(out=zts[b][:, lo:hi, :],
                                          in_=aff[img, b, :, lo:hi, :])

                # ---- PSUM accumulators (one bank each) ----
                psU = [pspool.tile([128, 2, W], FP32, tag="U", name=f"psU{_b}")
                       for _b in range(2)]
                psD = [pspool.tile([128, 2, W], FP32, tag="D", name=f"psD{_b}")
                       for _b in range(2)]
                psT = [pspool.tile([128, 2, W], FP32, tag="T", name=f"psT{_b}")
                       for _b in range(2)]
                started = set()

                def mm(acc, b, stop=False, **kw):
                    key = (acc, b)
                    nc.tensor.matmul(start=(key not in started), stop=stop,
                                     **kw)
                    started.add(key)

                dts = [dpool.tile([128, NTAP, W], BF16, tag="dt",
                                   name=f"dt{_b}") for _b in range(2)]
                # ring0 pre-folds for U (z) and D (d): 24 -> 12 planes
                zus = [fpool.tile([128, 12, W], BF16, tag="zu",
                                   name=f"zu{_b}") for _b in range(2)]
                dus = [fpool.tile([128, 12, W], BF16, tag="du",
                                   name=f"du{_b}") for _b in range(2)]

                # ---- per-chunk compute ----
                for ci, (lo, hi) in enumerate(DMA_CHUNKS):
                    ring = 0 if hi <= 24 else (1 if hi <= 40 else 2)
                    for b in range(2):
                        zt = zts[b]
                        zwin = zt[:, lo:hi, GW:GW + W]
                        # z = att_r * aff  (DVE, in place, bf16 2x)
                        nc.vector.tensor_tensor(
                            out=zwin, in0=zwin,
                            in1=attfs[b][:, ring:ring + 1, :].broadcast_to(
                                [128, hi - lo, W]),
                            op=mybir.AluOpType.mult)
                        # d = relu(-z): split DVE / POOL / ACT
                        dsl = dts[b][:, lo:hi, :]
                        nc.scalar.activation(
                            dsl, zwin, mybir.ActivationFunctionType.Relu,
                            scale=-1.0)

                    for b in range(2):
                        zt = zts[b]
                        if ci == 1:
                            # ring0 folds (need chunks 0+1): U and D inputs
                            nc.vector.tensor_tensor(
                                out=zus[b][:], in0=zt[:, 0:12, GW:GW + W],
                                in1=zt[:, 12:24, GW:GW + W],
                                op=mybir.AluOpType.add)
                            nc.vector.tensor_tensor(
                                out=dus[b][:], in0=dts[b][:, 0:12, :],
                                in1=dts[b][:, 12:24, :],
                                op=mybir.AluOpType.add)
                            for h in range(6):
                                mm("U", b, out=psU[b][:], lhsT=ident,
                                   rhs=zus[b][:, 2 * h:2 * h + 2, :])
                                mm("D", b, out=psD[b][:], lhsT=ident2,
                                   rhs=dus[b][:, 2 * h:2 * h + 2, :])
                        elif ci >= 2:
                            for t in range(lo, hi, 2):
                                mm("U", b, out=psU[b][:], lhsT=ident,
                                   rhs=zt[:, t:t + 2, GW:GW + W])
                                mm("D", b, stop=(t == 46),
                                   out=psD[b][:], lhsT=ident2,
                                   rhs=dts[b][:, t:t + 2, :])
                        # T matmuls for groups inside this chunk
                        for (g_lo, g_hi, i) in TGROUPS:
                            if g_lo < lo or g_lo >= hi:
                                continue
                            dy = 3 - i
                            lw = bandt[:, C0 + dy:C0 + dy + 128]
                            t = g_lo
                            while t + 1 < g_hi:
                                mm("T", b, out=psT[b][:], lhsT=lw,
                                   rhs=_pair_ap(zt, t, t + 1,
                                                GW + dxof(t), GW + dxof(t + 1)))
                                t += 2
                            if t < g_hi:
                                mm("T", b, out=psT[b][:, 0, :], lhsT=lw,
                                   rhs=_single_ap(zt, t, GW + dxof(t)))
                            # halo: b0 rows need dy>0 from b1; b1 need dy<0;
                            # emitted at b==1 so both blocks' z is ready
                            if b == 1 and dy != 0:
                                if dy > 0:
                                    hw = bandt[:, 3 + dy:3 + dy + 128]
                                    dst, other = 0, zts[1]
                                else:
                                    hw = bandt[:, 259 + dy:259 + dy + 128]
                                    dst, other = 1, zts[0]
                                t = g_lo
                                while t + 1 < g_hi:
                                    mm("T", dst, out=psT[dst][:], lhsT=hw,
                                       rhs=_pair_ap(other, t, t + 1,
                                                    GW + dxof(t),
                                                    GW + dxof(t + 1)))
                                    t += 2
                                if t < g_hi:
                                    mm("T", dst, out=psT[dst][:, 0, :],
                                       lhsT=hw,
                                       rhs=_single_ap(other, t, GW + dxof(t)))

                # closers: psU += att3, psT += att3 (stop their groups)
                for b in range(2):
                    mm("U", b, stop=True, out=psU[b][:, 0, :], lhsT=ident,
                       rhs=attfs[b][:, 3, :])
                    mm("T", b, stop=True, out=psT[b][:, 0, :], lhsT=ident,
                       rhs=attfs[b][:, 3, :])

                # ---- epilogue ----
                for b in range(2):
                    # DVE can read only one PSUM operand per op: stage the
                    # second accumulator halves through SBUF on ACT
                    u1 = epool.tile([128, W], FP32, tag="u1")
                    nc.scalar.copy(u1[:], psU[b][:, 1, :])
                    d1 = epool.tile([128, W], FP32, tag="d1")
                    nc.scalar.copy(d1[:], psD[b][:, 1, :])
                    t1 = epool.tile([128, W], FP32, tag="t1")
                    nc.scalar.copy(t1[:], psT[b][:, 1, :])
                    # in-place accumulations to keep the tile count low
                    nc.vector.tensor_tensor(out=u1[:], in0=psU[b][:, 0, :],
                                            in1=u1[:],
                                            op=mybir.AluOpType.add)
                    nc.vector.tensor_tensor(out=d1[:], in0=psD[b][:, 0, :],
                                            in1=d1[:],
                                            op=mybir.AluOpType.add)
                    e = epool.tile([128, W], FP32, tag="e")
                    nc.vector.scalar_tensor_tensor(
                        out=e[:], in0=u1[:], scalar=EPS, in1=d1[:],
                        op0=mybir.AluOpType.add, op1=mybir.AluOpType.add)
                    rcp = epool.tile([128, W], FP32, tag="rcp")
                    nc.vector.reciprocal_approx_fast(out=rcp[:], in_=e[:])
                    nc.vector.tensor_tensor(out=d1[:], in0=d1[:],
                                            in1=cots[b][:],
                                            op=mybir.AluOpType.mult)
                    nc.vector.tensor_tensor(out=t1[:], in0=psT[b][:, 0, :],
                                            in1=t1[:],
                                            op=mybir.AluOpType.add)
                    nc.vector.tensor_tensor(out=t1[:], in0=t1[:],
                                            in1=csts[b][:],
                                            op=mybir.AluOpType.mult)
                    nc.vector.tensor_tensor(out=t1[:], in0=t1[:],
                                            in1=d1[:],
                                            op=mybir.AluOpType.add)
                    outt = epool.tile([128, W], FP32, tag="outt")
                    nc.vector.tensor_tensor(out=outt[:], in0=t1[:],
                                            in1=rcp[:],
                                            op=mybir.AluOpType.mult)
                    nc.sync.dma_start(
                        out=out[img, 0, b * 128:b * 128 + 128, :],
                        in_=outt[:])

    nc.compile()
    return nc


_NC_CACHE = None


def _get_nc():
    global _NC_CACHE
    if _NC_CACHE is None:
        _NC_CACHE = _build()
    return _NC_CACHE


def run(inputs: dict, trace: bool = False):
    aff = np.ascontiguousarray(np.asarray(inputs["affinity"], dtype=np.float32))
    att = np.ascontiguousarray(np.asarray(inputs["attention"], dtype=np.float32))
    cs = np.ascontiguousarray(
        np.asarray(inputs["current_segmentation"], dtype=np.float32))
    co = np.ascontiguousarray(
        np.asarray(inputs["coarse_segmentation"], dtype=np.float32))
    in_maps = pack_inputs(aff, att, cs, co)

    nc = _get_nc()
    last_err = None
    for attempt in range(3):
        try:
            res = run_bass_kernel_spmd(nc, in_maps, list(range(N_CORES)),
                                       trace=trace)
            break
        except Exception as e:
            last_err = e
            import time
            time.sleep(10)
    else:
        raise last_err
    full = np.concatenate([res.results[c]["out"] for c in range(N_CORES)],
                          axis=0)
    return full, res


def kernel(**inputs) -> np.ndarray:
    out, _ = run(inputs, trace=False)
    return out
